# revision 16
# baseline (speedup 1.0000x reference)
"""Trainium2 Bass kernel for MultiHead GQA attention (B=2, S=2048, D=2048,
H=16 query heads, HKV=4 kv heads, DH=128, RoPE, mask, out-proj).

Sharding: token-parallel across 8 cores. Core c handles batch c//4 and 512
query rows of it (4 blocks of 128 rows). Each core projects K/V for its own
512-token quarter, all-gathers projected K/V across the 4 cores of its batch
(split into a K collective and a V collective so the gather starts as early
as possible), projects Q for its rows, runs attention + out-proj for its
rows, and writes its [512, 2048] slice in bf16 (host casts to fp32).

All matmuls run in bf16 with fp32 PSUM accumulation. Host pre-transposes /
pre-tiles every operand so each DMA is a contiguous [128, X] block and each
matmul consumes operands with the contraction dim on partitions.

Attention is computed transposed, two heads of a KV group at a time:
scoresT[keys, q] = khT.T @ qhT per 128-key tile, exp on ScalarE (scale
folded in), probs stored bf16, outT[dh, q] += v_tile.T @ probsT. Row sums
accumulate into a single shared [4, 512] PSUM bank per head-group via
one-hot [128, 4] stationaries, so softmax denominators cost one bank total.

Mask handling (host-detected, compile-time mode):
  none   - mask has no zeros: no mask work at all.
  causal - mask is exactly tril: balanced interleaved q-blocks per core +
           suffix key-ranges (only ~62% of attention tiles computed). Only
           the lowest 128-q block of each suffix can be masked (it is
           always tril or all-zero), so probs are multiplied by a narrow
           [128, 128] per-key-tile mask.
  mask   - anything else: all tiles computed, probs multiplied by 0/1 mask.
"""

import math

import numpy as np
import ml_dtypes

import concourse.bass as bass
import concourse.mybir as mybir
import concourse.tile as tile
from concourse import bacc
from concourse.bass_utils import run_bass_kernel_spmd

F32 = mybir.dt.float32
BF16 = mybir.dt.bfloat16
BF = ml_dtypes.bfloat16

B, S, D = 2, 2048, 2048
H, G = 16, 4
HKV = H // G            # 4
DH = D // H             # 128
DKV = D // G            # 512 (kv projection width)
NCORES = 8
RPC = S // 4            # 512 rows per core
NQB = RPC // 128        # 4 q-blocks of 128 rows per core
NIC = D // 128          # 16 contraction chunks
NKC = S // 128          # 16 key tiles
SCALE = 1.0 / math.sqrt(DH)

_NC_CACHE: dict = {}

# set by callers (e.g. test.py) to capture a profile; results of the last run
TRACE = False
TRACE_CORES = None          # e.g. [0] or list(range(8))
LAST_RESULTS = None


def _n_list(mode: str) -> list[int]:
    """Moving-operand width (in q columns, suffix of the 512) per key tile."""
    if mode == "causal":
        # per key-tile kc, every core keeps exactly (4 - kc//4) of its 4
        # interleaved q-blocks {r, 7-r, 8+r, 15-r} (ascending order)
        return [128 * (4 - kc // 4) for kc in range(NKC)]
    return [512] * NKC


def _build(mode: str):
    mask_mul = mode != "none"
    n_list = _n_list(mode)

    nc = bacc.Bacc("TRN2", target_bir_lowering=False, debug=False,
                   num_devices=NCORES)

    # ---- I/O (host-prepared layouts; all contiguous-DMA friendly) ----
    wq = nc.declare_dram_parameter("wq", [NIC, 128, D], BF16, isOutput=False)
    qt = nc.declare_dram_parameter("qt", [128, NIC * RPC], BF16, isOutput=False)
    # k/v: only this core's 512-token quarter (projected here, all-gathered)
    kt = nc.declare_dram_parameter("kt", [128, NIC * 512], BF16, isOutput=False)
    vt = nc.declare_dram_parameter("vt", [4, 128, NIC * 128], BF16, isOutput=False)
    wk = nc.declare_dram_parameter("wk", [HKV, 128, NIC * 128], BF16, isOutput=False)
    wv = nc.declare_dram_parameter("wv", [128, NIC * DKV], BF16, isOutput=False)
    wo = nc.declare_dram_parameter("wo", [4, 128, H * 512], BF16, isOutput=False)
    cosq = nc.declare_dram_parameter("cosq", [128, RPC], BF16, isOutput=False)
    sinq = nc.declare_dram_parameter("sinq", [128, RPC], BF16, isOutput=False)
    # cos/sin for this core's own k-token quarter
    cosk = nc.declare_dram_parameter("cosk", [128, 512], BF16, isOutput=False)
    sink = nc.declare_dram_parameter("sink", [128, 512], BF16, isOutput=False)
    pswap = nc.declare_dram_parameter("pswap", [128, 128], BF16, isOutput=False)
    # one-hot columns for the shared row-sum bank: ehot[:, 4h+j] = (j == h)
    ehot = nc.declare_dram_parameter("ehot", [128, 16], BF16, isOutput=False)
    if mask_mul:
        mw = 128 if mode == "causal" else RPC
        m01 = nc.declare_dram_parameter("m01", [128, NKC * mw], BF16,
                                        isOutput=False)
    out = nc.declare_dram_parameter("out", [RPC, D], BF16, isOutput=True)

    with tile.TileContext(nc) as tc:
        with (
            tc.tile_pool(name="res", bufs=1) as res,          # resident
            tc.tile_pool(name="stream2m", bufs=2) as stream2m,  # 2MB blocks
            tc.tile_pool(name="stream05", bufs=6) as stream05,  # 0.5MB blocks
            tc.tile_pool(name="small", bufs=3) as small,
            tc.tile_pool(name="probs", bufs=4) as probsp,
            tc.tile_pool(name="bcast", bufs=2) as bcastp,
            tc.tile_pool(name="dram", bufs=1, space="DRAM") as dramp,
            tc.tile_pool(name="psmm", bufs=3, space="PSUM") as psmm,
            tc.tile_pool(name="psacc", bufs=2, space="PSUM") as psacc,
            tc.tile_pool(name="pssum", bufs=1, space="PSUM") as pssum,
        ):
            # ---------------- resident tiles (DMAs staged per phase) -------
            # K-path first so the first matmul isn't stuck behind bulk loads
            coskq_t = res.tile([128, 512], BF16)
            nc.sync.dma_start(out=coskq_t, in_=cosk[:, :])
            sinkq_t = res.tile([128, 512], BF16)
            nc.sync.dma_start(out=sinkq_t, in_=sink[:, :])
            pswap_t = res.tile([128, 128], BF16)
            nc.sync.dma_start(out=pswap_t, in_=pswap[:, :])

            # allocated here (tag order: qts before outu_a), loaded later
            qts = res.tile([128, NIC, RPC], BF16)

            qhs = res.tile([128, H, RPC], BF16)     # rope'd q, [dh, h, rows]
            khs = res.tile([128, HKV, S], BF16)     # rope'd k, [dh, hk, keys]
            vhs = res.tile([128, 16, DKV], BF16)    # v heads, [tok%128, tokc, kv]
            # outu_a shares qts's slot: qts is dead once phase A finishes.
            # split 12/4 so phase D's early matmuls (h<12) don't dep-chain
            # behind the last normalization group (h>=12).
            outu_a = res.tile([128, 12, RPC], BF16, tag="qts")
            outu_b = res.tile([128, 4, RPC], BF16)

            def outu(h):
                return outu_a[:, h, :] if h < 12 else outu_b[:, h - 12, :]
            rec_dram = dramp.tile([16, RPC], F32)
            khs_own = res.tile([128, HKV, 512], BF16)
            vhs_own = res.tile([128, 4, DKV], BF16)
            k_own = dramp.tile([128, HKV, 512], BF16)
            v_own = dramp.tile([128, 4, DKV], BF16)
            k_all = dramp.tile([4, 128, HKV, 512], BF16)
            v_all = dramp.tile([4, 128, 4, DKV], BF16)

            def rope(dst, x_bf, ps_pool, cos_ap, sin_ap, n):
                """dst = x*cos + pairswap(x)*sin  (signs baked into sin)."""
                y_ps = ps_pool.tile([128, 512], F32, tag="mm")
                # moving operand max 1024 bf16 per matmul
                assert n <= 512
                nc.tensor.matmul(y_ps[:, :n], pswap_t, x_bf, start=True,
                                 stop=True)
                t1 = small.tile([128, 512], BF16, tag="t1")
                nc.vector.tensor_mul(t1[:, :n], x_bf, cos_ap)
                t2 = small.tile([128, 512], BF16, tag="t2")
                nc.vector.tensor_mul(t2[:, :n], y_ps[:, :n], sin_ap)
                nc.vector.tensor_add(dst, t1[:, :n], t2[:, :n])

            # ------- Phase B1: K proj for OWN 512-token quarter + RoPE -----
            # (first, so the K all-gather overlaps V proj + Q proj below)
            # wk + kmov loaded in interleaved 0.5MB chunks so the first
            # matmuls start after ~1MB instead of after the full 4MB.
            # Later-phase loads are issued from the Vector queue mid-K-proj
            # so the K path gets the full DMA bandwidth at t=0.
            kmov = stream2m.tile([128, NIC, 512], BF16, tag="s2m")
            wks = res.tile([128, HKV, NIC * 128], BF16)
            for c in range(4):
                nc.sync.dma_start(out=wks[:, c, :], in_=wk[c])
                nc.sync.dma_start(
                    out=kmov[:, 4 * c:4 * c + 4, :],
                    in_=kt[:, 4 * c * 512:(4 * c + 4) * 512].rearrange(
                        "p (i m) -> p i m", i=4))
            ehot_t = res.tile([128, 16], BF16)
            nc.sync.dma_start(out=ehot_t, in_=ehot[:, :])
            # touch Exp early so the ~2.7us ACT table load is off the
            # attention critical path
            warm = res.tile([1, 16], BF16)
            nc.scalar.activation(warm, ehot_t[:1, :],
                                 mybir.ActivationFunctionType.Exp)
            wvs = res.tile([128, NIC, DKV], BF16)
            vmovs = []
            for j in range(4):
                vmov = stream05.tile([128, NIC, 128], BF16, tag="s05",
                                     name="vmov")
                vmovs.append(vmov)
            cosq_t = res.tile([128, RPC], BF16)
            sinq_t = res.tile([128, RPC], BF16)
            wq_tiles = []
            for oc in range(2):
                wq_all = stream05.tile([128, NIC, 128], BF16, tag="s05",
                                       name="wq_all")
                wq_tiles.append(wq_all)
            # cos/sin for own k-token quarter live in coskq (host-sliced)
            # rope for hk runs after hk+1's matmuls so the rope matmul
            # (which waits on the ScalarE copy) never blocks the PE FIFO
            pend_k = None
            for hk in range(HKV):
                wk_all = wks[:, hk, :].rearrange("p (i m) -> p i m", i=NIC)
                ps = psmm.tile([128, 512], F32, tag="mm")
                for ic in range(NIC):
                    nc.tensor.matmul(ps, wk_all[:, ic, :],
                                     kmov[:, ic, :],
                                     start=(ic == 0), stop=(ic == NIC - 1))
                xk = small.tile([128, 512], BF16, tag="xq")
                nc.scalar.copy(xk, ps)
                if hk == 1:
                    # V-path loads, issued once the K path is streaming
                    nc.scalar.dma_start(out=wvs, in_=wv[:, :].rearrange(
                        "p (i n) -> p i n", i=NIC))
                    for j in range(4):
                        nc.scalar.dma_start(out=vmovs[j], in_=vt[j].rearrange(
                            "p (i m) -> p i m", i=NIC))
                elif hk == 3:
                    # Q-path loads
                    for c in range(4):
                        nc.scalar.dma_start(
                            out=qts[:, 4 * c:4 * c + 4, :],
                            in_=qt[:, 4 * c * 512:(4 * c + 4) * 512]
                            .rearrange("p (i m) -> p i m", i=4))
                    nc.scalar.dma_start(out=cosq_t, in_=cosq[:, :])
                    nc.scalar.dma_start(out=sinq_t, in_=sinq[:, :])
                    for oc in range(2):
                        nc.scalar.dma_start(
                            out=wq_tiles[oc], in_=wq[oc].rearrange(
                                "p (i m) -> p i m", i=NIC))
                if pend_k is not None:
                    rope(khs_own[:, pend_k[0], :], pend_k[1], psmm,
                         coskq_t, sinkq_t, 512)
                pend_k = (hk, xk)
            rope(khs_own[:, pend_k[0], :], pend_k[1], psmm,
                 coskq_t, sinkq_t, 512)

            # ---- all-gather projected K across the 4 cores of the batch --
            nc.gpsimd.dma_start(out=k_own, in_=khs_own)
            nc.gpsimd.collective_compute(
                "AllGather", mybir.AluOpType.bypass,
                replica_groups=[[0, 1, 2, 3], [4, 5, 6, 7]],
                ins=[k_own[:, :, :]], outs=[k_all[:, :, :, :]])
            # land gathered K via the otherwise-idle gpsimd queue: these
            # wait on the collective and must not HOL-block input loads
            for r in range(4):
                nc.gpsimd.dma_start(out=khs[:, :, r * 512:(r + 1) * 512],
                                    in_=k_all[r])

            # ------- Phase B2: V proj for OWN quarter + all-gather ---------
            for j in range(4):            # own 128-token blocks (V stationary)
                vmov = vmovs[j]
                ps = psmm.tile([128, 512], F32, tag="mm")
                for ic in range(NIC):
                    nc.tensor.matmul(ps, vmov[:, ic, :],
                                     wvs[:, ic, :],
                                     start=(ic == 0), stop=(ic == NIC - 1))
                nc.vector.tensor_copy(vhs_own[:, j, :], ps)

            nc.gpsimd.dma_start(out=v_own, in_=vhs_own)
            nc.gpsimd.collective_compute(
                "AllGather", mybir.AluOpType.bypass,
                replica_groups=[[0, 1, 2, 3], [4, 5, 6, 7]],
                ins=[v_own[:, :, :]], outs=[v_all[:, :, :, :]])
            for r in range(4):
                nc.gpsimd.dma_start(out=vhs[:, 4 * r:4 * r + 4, :],
                                    in_=v_all[r])

            # ---------------- Phase A: Q-proj + RoPE ----------------
            pend_q = None
            for oc in range(H):
                if oc < 2:
                    wq_all = wq_tiles[oc]
                else:
                    wq_all = stream05.tile([128, NIC, 128], BF16, tag="s05")
                    nc.sync.dma_start(out=wq_all, in_=wq[oc].rearrange(
                        "p (i m) -> p i m", i=NIC))
                ps = psmm.tile([128, 512], F32, tag="mm")
                for ic in range(NIC):
                    nc.tensor.matmul(ps, wq_all[:, ic, :],
                                     qts[:, ic, :],
                                     start=(ic == 0), stop=(ic == NIC - 1))
                xq = small.tile([128, 512], BF16, tag="xq")
                nc.scalar.copy(xq, ps)
                if pend_q is not None:
                    rope(qhs[:, pend_q[0], :], pend_q[1], psmm,
                         cosq_t, sinq_t, RPC)
                pend_q = (oc, xq)
            rope(qhs[:, pend_q[0], :], pend_q[1], psmm, cosq_t, sinq_t, RPC)

            # ---------------- Phase C: attention, 2 heads at a time -------
            if mask_mul:
                mwid = 128 if mode == "causal" else RPC
                m01s = res.tile([128, NKC, mwid], BF16)
                nc.sync.dma_start(out=m01s, in_=m01[:, :].rearrange(
                    "p (k m) -> p k m", k=NKC))

            def normalize_group(g, sm):
                """reciprocal + broadcast + in-place normalize for the 4
                heads of group g, given their sums in SBUF sm [4, RPC]."""
                rec = small.tile([4, RPC], F32, tag="rec", bufs=2)
                nc.vector.reciprocal_approx_fast(rec, sm)
                nc.sync.dma_start(out=rec_dram[4 * g:4 * g + 4, :], in_=rec)
                for j in range(4):
                    h = 4 * g + j
                    recb = bcastp.tile([128, RPC], F32, tag="bc")
                    nc.sync.dma_start(
                        out=recb,
                        in_=rec_dram[h:h + 1, :].to_broadcast([128, RPC]))
                    nc.vector.tensor_mul(outu(h), outu(h), recb)

            for g in range(HKV):
                hk = g
                ps_s = pssum.tile([4, 512], F32, tag="sum")
                for pair in range(2):
                    h0 = 4 * g + 2 * pair
                    ps_o0 = psacc.tile([128, 512], F32, tag="acc")
                    ps_o1 = psacc.tile([128, 512], F32, tag="acc2")

                    def sums_av(kc, probs):
                        n = n_list[kc]
                        lo = RPC - n
                        first = kc == 0 and pair == 0
                        last = kc == NKC - 1 and pair == 1
                        for j in range(2):
                            h = h0 + j
                            nc.tensor.matmul(
                                ps_s[:, lo:], ehot_t[:, 4 * (h - 4 * g):
                                                     4 * (h - 4 * g) + 4],
                                probs[:, j, :n],
                                start=(first and j == 0),
                                stop=(last and j == 1),
                                skip_group_check=True)
                            nc.tensor.matmul(
                                (ps_o0 if j == 0 else ps_o1)[:, lo:],
                                vhs[:, kc, hk * 128:(hk + 1) * 128],
                                probs[:, j, :n],
                                start=(kc == 0), stop=(kc == NKC - 1),
                                skip_group_check=True)

                    # sums/AV for key tile kc issue after kc+1's scores so
                    # the PE never waits on exp/mask of the current tile
                    pend = None
                    for kc in range(NKC):
                        n = n_list[kc]
                        lo = RPC - n          # suffix columns
                        probs = probsp.tile([128, 2, 512], BF16, tag="pr")
                        for j in range(2):
                            h = h0 + j
                            ps_sc = psmm.tile([128, 512], F32, tag="mm")
                            nc.tensor.matmul(
                                ps_sc[:, :n],
                                khs[:, hk, kc * 128:(kc + 1) * 128],
                                qhs[:, h, lo:],
                                start=True, stop=True, skip_group_check=True)
                            nc.scalar.activation(
                                probs[:, j, :n], ps_sc[:, :n],
                                mybir.ActivationFunctionType.Exp, scale=SCALE)
                        if mask_mul:
                            if mode == "causal":
                                # only the lowest 128-q block of the suffix
                                # can be masked (tril diagonal or all-zero)
                                for j in range(2):
                                    nc.vector.tensor_mul(
                                        probs[:, j, :128],
                                        probs[:, j, :128],
                                        m01s[:, kc, :])
                            else:
                                for j in range(2):
                                    nc.vector.tensor_mul(
                                        probs[:, j, :n], probs[:, j, :n],
                                        m01s[:, kc, lo:])
                        if pend is not None:
                            sums_av(*pend)
                        pend = (kc, probs)
                    sums_av(*pend)
                    nc.vector.tensor_copy(outu(h0), ps_o0)
                    nc.vector.tensor_copy(outu(h0 + 1), ps_o1)
                sm = small.tile([4, RPC], F32, tag="sm4", bufs=2)
                nc.vector.tensor_copy(sm, ps_s)
                normalize_group(g, sm)

            # ---------------- Phase D: out-projection ----------------
            for oc in range(4):
                wo_all = stream2m.tile([128, H, 512], BF16, tag="s2m")
                nc.sync.dma_start(out=wo_all, in_=wo[oc].rearrange(
                    "p (h m) -> p h m", h=H))
                for qc in range(NQB):
                    if qc % 2:
                        ps_f = psmm.tile([128, 512], F32, tag="mm",
                                         name="ps_f")
                    else:
                        ps_f = psacc.tile([128, 512], F32, tag="acc",
                                          name="ps_f")
                    for h in range(H):
                        lh = outu_a[:, h, qc * 128:(qc + 1) * 128] if h < 12 \
                            else outu_b[:, h - 12, qc * 128:(qc + 1) * 128]
                        nc.tensor.matmul(
                            ps_f, lh, wo_all[:, h, :],
                            start=(h == 0), stop=(h == H - 1))
                    fin = small.tile([128, 512], BF16, tag="fin")
                    nc.vector.tensor_copy(fin, ps_f)
                    nc.sync.dma_start(
                        out=out[qc * 128:(qc + 1) * 128,
                                oc * 512:(oc + 1) * 512],
                        in_=fin)

    nc.compile()
    return nc


def _get_nc(mode: str):
    if mode not in _NC_CACHE:
        _NC_CACHE[mode] = _build(mode)
    return _NC_CACHE[mode]


def _core_rows(mode: str, r: int) -> np.ndarray:
    """Global (within-batch) q-row indices owned by quarter r, ascending."""
    if mode == "causal":
        blocks = sorted([r, 7 - r, 8 + r, 15 - r])
    else:
        blocks = [4 * r, 4 * r + 1, 4 * r + 2, 4 * r + 3]
    return np.concatenate([np.arange(b * 128, (b + 1) * 128) for b in blocks])


def kernel(q, k, v, mask, freqs, W_q, W_k, W_v, W_o):
    q = np.asarray(q, dtype=np.float32)
    k = np.asarray(k, dtype=np.float32)
    v = np.asarray(v, dtype=np.float32)
    mask = np.asarray(mask, dtype=np.float32)
    freqs = np.asarray(freqs, dtype=np.float32)
    W_q = np.asarray(W_q, dtype=np.float32)
    W_k = np.asarray(W_k, dtype=np.float32)
    W_v = np.asarray(W_v, dtype=np.float32)
    W_o = np.asarray(W_o, dtype=np.float32)

    # ---- mask mode detection ----
    nz = mask != 0
    if nz.all():
        mode = "none"
    else:
        tril = np.tril(np.ones((S, S), dtype=bool))
        mode = "causal" if all(np.array_equal(nz[b], tril) for b in range(B)) \
            else "mask"

    # ---- shared host precomputation ----
    c_full = np.cos(freqs)                      # [S, 64]
    s_full = np.sin(freqs)
    sgn = np.tile(np.array([-1.0, 1.0], np.float32), DH // 2)  # [-,+,-,+...]
    cosk_h = np.repeat(c_full, 2, axis=1).T.astype(BF)          # [128, S]
    sink_h = (np.repeat(s_full, 2, axis=1) * sgn).T.astype(BF)

    psw = np.zeros((128, 128), np.float32)
    idx = np.arange(128)
    psw[idx, idx ^ 1] = 1.0
    psw = psw.astype(BF)

    eh = np.zeros((128, 16), np.float32)
    for h in range(4):
        eh[:, 4 * h + h] = 1.0
    eh = eh.astype(BF)

    # weight layouts
    # wq[oc, p, i*128+m] = W_q[oc*128+m, i*128+p]
    wq_h = np.ascontiguousarray(
        W_q.reshape(H, 128, NIC, 128).transpose(0, 3, 2, 1)
        .reshape(H, 128, D)).astype(BF)
    # wk[hk, p, i*128+m] = W_k[hk*128+m, i*128+p]
    wk_h = np.ascontiguousarray(
        W_k.reshape(HKV, 128, NIC, 128).transpose(0, 3, 2, 1)
        .reshape(HKV, 128, D)).astype(BF)
    # wv[p, i*512+n] = W_v[n, i*128+p]
    wv_h = np.ascontiguousarray(
        W_v.reshape(DKV, NIC, 128).transpose(2, 1, 0).reshape(128, NIC * DKV)
    ).astype(BF)
    # wo[oc, p, h*512+m] = W_o[oc*512+m, h*128+p]
    wo_h = np.ascontiguousarray(
        W_o.reshape(4, 512, H, 128).transpose(0, 3, 2, 1).reshape(4, 128, -1)
    ).astype(BF)

    # k/v: each core only gets its own 512-token quarter (gathered on device)
    # kt[p, i*512+t] = k[b, tq*512+t, i*128+p] for quarter tq
    kt_b = []   # [B][4] quarters
    vt_b = []
    for b in range(B):
        kt_b.append([np.ascontiguousarray(
            k[b, tq * 512:(tq + 1) * 512].reshape(512, NIC, 128)
            .transpose(2, 1, 0).reshape(128, NIC * 512)).astype(BF)
            for tq in range(4)])
        # vt[j, p, i*128+t] = v[b, tq*512 + j*128+t, i*128+p]
        vt_b.append([np.ascontiguousarray(
            v[b, tq * 512:(tq + 1) * 512].reshape(4, 128, NIC, 128)
            .transpose(0, 3, 2, 1).reshape(4, 128, NIC * 128)).astype(BF)
            for tq in range(4)])

    in_maps = []
    rows_all = []
    for c in range(NCORES):
        b, r = divmod(c, 4)
        rows = _core_rows(mode, r)
        rows_all.append((b, rows))
        # qt[p, i*512+t] = q[b, rows[t], i*128+p]
        qsl = q[b][rows]                       # [512, D]
        qt_h = np.ascontiguousarray(
            qsl.reshape(RPC, NIC, 128).transpose(2, 1, 0).reshape(128, -1)
        ).astype(BF)
        cq = np.repeat(c_full[rows], 2, axis=1).T.astype(BF)      # [128, 512]
        sq = (np.repeat(s_full[rows], 2, axis=1) * sgn).T.astype(BF)
        im = {
            "wq": wq_h, "qt": qt_h, "kt": kt_b[b][r], "vt": vt_b[b][r],
            "wk": wk_h, "wv": wv_h, "wo": wo_h,
            "cosq": cq, "sinq": sq,
            "cosk": np.ascontiguousarray(cosk_h[:, r * 512:(r + 1) * 512]),
            "sink": np.ascontiguousarray(sink_h[:, r * 512:(r + 1) * 512]),
            "pswap": psw, "ehot": eh,
        }
        if mode == "causal":
            # narrow mask: per key tile kc, the 0/1 mask of the LOWEST
            # 128-q block of this core's kept suffix (tril, zero, or ones)
            n_l = _n_list(mode)
            m01_h = np.empty((NKC, 128, 128), np.float32)
            for kc in range(NKC):
                lo = RPC - n_l[kc]
                qrows = rows[lo:lo + 128]          # global q rows of block
                kcols = np.arange(kc * 128, (kc + 1) * 128)
                m01_h[kc] = (qrows[None, :] >= kcols[:, None])  # [k, q]
            im["m01"] = np.ascontiguousarray(
                m01_h.transpose(1, 0, 2).reshape(128, -1)).astype(BF)
        elif mode == "mask":
            # m01[p, kc*512+m] = (mask[b, rows[m], kc*128+p] != 0)
            msl = nz[b][rows]                  # [512, S] bool
            m01_h = np.ascontiguousarray(
                msl.T.reshape(NKC, 128, RPC).transpose(1, 0, 2)
                .reshape(128, -1)).astype(BF)
            im["m01"] = m01_h
        in_maps.append(im)

    nc = _get_nc(mode)
    kwargs = {}
    if TRACE:
        kwargs["trace"] = True
        if TRACE_CORES:
            kwargs["trace_cores"] = list(TRACE_CORES)
    results = run_bass_kernel_spmd(nc, in_maps, core_ids=list(range(NCORES)),
                                   **kwargs)
    global LAST_RESULTS
    LAST_RESULTS = results

    full = np.empty((B, S, D), np.float32)
    for c in range(NCORES):
        b, rows = rows_all[c]
        full[b, rows] = results.results[c]["out"].astype(np.float32)
    return full


# revision 21
# speedup vs baseline: 1.3920x; 1.3920x over previous
"""Trainium2 Bass kernel for MultiHead GQA attention (B=2, S=2048, D=2048,
H=16 query heads, HKV=4 kv heads, DH=128, RoPE, mask, out-proj).

Sharding: token-parallel across 8 cores. Core c handles batch c//4 and 512
query rows of it (4 blocks of 128 rows). Each core projects K/V for its own
512-token quarter, all-gathers projected K/V across the 4 cores of its batch
(split into a K collective and a V collective so the gather starts as early
as possible), projects Q for its rows, runs attention + out-proj for its
rows, and writes its [512, 2048] slice in bf16 (host casts to fp32).

All matmuls run in bf16 with fp32 PSUM accumulation. Host pre-transposes /
pre-tiles every operand so each DMA is a contiguous [128, X] block and each
matmul consumes operands with the contraction dim on partitions.

Attention is computed transposed, two heads of a KV group at a time:
scoresT[keys, q] = khT.T @ qhT per 128-key tile, exp on ScalarE (scale
folded in), probs stored bf16, outT[dh, q] += v_tile.T @ probsT. Row sums
accumulate into a single shared [4, 512] PSUM bank per head-group via
one-hot [128, 4] stationaries, so softmax denominators cost one bank total.

Mask handling (host-detected, compile-time mode):
  none   - mask has no zeros: no mask work at all.
  causal - mask is exactly tril: balanced interleaved q-blocks per core +
           suffix key-ranges (only ~62% of attention tiles computed). Only
           the lowest 128-q block of each suffix can be masked (it is
           always tril or all-zero), so probs are multiplied by a narrow
           [128, 128] per-key-tile mask.
  mask   - anything else: all tiles computed, probs multiplied by 0/1 mask.
"""

import math

import numpy as np
import ml_dtypes

import concourse.bass as bass
import concourse.mybir as mybir
import concourse.tile as tile
from concourse import bacc
from concourse.bass_utils import run_bass_kernel_spmd

F32 = mybir.dt.float32
BF16 = mybir.dt.bfloat16
BF = ml_dtypes.bfloat16

B, S, D = 2, 2048, 2048
H, G = 16, 4
HKV = H // G            # 4
DH = D // H             # 128
DKV = D // G            # 512 (kv projection width)
NCORES = 8
RPC = S // 4            # 512 rows per core
NQB = RPC // 128        # 4 q-blocks of 128 rows per core
NIC = D // 128          # 16 contraction chunks
NKC = S // 128          # 16 key tiles
SCALE = 1.0 / math.sqrt(DH)

_NC_CACHE: dict = {}

# set by callers (e.g. test.py) to capture a profile; results of the last run
TRACE = False
TRACE_CORES = None          # e.g. [0] or list(range(8))
LAST_RESULTS = None


def _n_list(mode: str) -> list[int]:
    """Moving-operand width (in q columns, suffix of the 512) per key tile."""
    if mode == "causal":
        # per key-tile kc, every core keeps exactly (4 - kc//4) of its 4
        # interleaved q-blocks {r, 7-r, 8+r, 15-r} (ascending order)
        return [128 * (4 - kc // 4) for kc in range(NKC)]
    return [512] * NKC


def _build(mode: str):
    mask_mul = mode != "none"
    n_list = _n_list(mode)

    nc = bacc.Bacc("TRN2", target_bir_lowering=False, debug=False,
                   num_devices=NCORES)

    # ---- I/O (host-prepared layouts; all contiguous-DMA friendly) ----
    wq = nc.declare_dram_parameter("wq", [NIC, 128, D], BF16, isOutput=False)
    qt = nc.declare_dram_parameter("qt", [128, NIC * RPC], BF16, isOutput=False)
    # k/v: only this core's 512-token quarter (projected here, all-gathered)
    kt = nc.declare_dram_parameter("kt", [128, NIC * 512], BF16, isOutput=False)
    vt = nc.declare_dram_parameter("vt", [4, 128, NIC * 128], BF16, isOutput=False)
    wk = nc.declare_dram_parameter("wk", [HKV, 128, NIC * 128], BF16, isOutput=False)
    wv = nc.declare_dram_parameter("wv", [128, NIC * DKV], BF16, isOutput=False)
    wo = nc.declare_dram_parameter("wo", [4, 128, H * 512], BF16, isOutput=False)
    cosq = nc.declare_dram_parameter("cosq", [128, RPC], BF16, isOutput=False)
    sinq = nc.declare_dram_parameter("sinq", [128, RPC], BF16, isOutput=False)
    # cos/sin for this core's own k-token quarter
    cosk = nc.declare_dram_parameter("cosk", [128, 512], BF16, isOutput=False)
    sink = nc.declare_dram_parameter("sink", [128, 512], BF16, isOutput=False)
    pswap = nc.declare_dram_parameter("pswap", [128, 128], BF16, isOutput=False)
    # one-hot columns for the shared row-sum bank: ehot[:, 4h+j] = (j == h)
    ehot = nc.declare_dram_parameter("ehot", [128, 16], BF16, isOutput=False)
    if mask_mul:
        mw = 128 if mode == "causal" else RPC
        m01 = nc.declare_dram_parameter("m01", [128, NKC * mw], BF16,
                                        isOutput=False)
    out = nc.declare_dram_parameter("out", [RPC, D], BF16, isOutput=True)

    with tile.TileContext(nc) as tc:
        with (
            tc.tile_pool(name="res", bufs=1) as res,          # resident
            tc.tile_pool(name="stream2m", bufs=3) as stream2m,  # 1MB halves
            tc.tile_pool(name="stream05", bufs=5) as stream05,  # 0.5MB blocks
            tc.tile_pool(name="small", bufs=3) as small,
            tc.tile_pool(name="probs", bufs=5) as probsp,
            tc.tile_pool(name="bcast", bufs=2) as bcastp,
            tc.tile_pool(name="dram", bufs=1, space="DRAM") as dramp,
            tc.tile_pool(name="psmm", bufs=3, space="PSUM") as psmm,
            tc.tile_pool(name="psacc", bufs=2, space="PSUM") as psacc,
            tc.tile_pool(name="pssum", bufs=1, space="PSUM") as pssum,
        ):
            # ---------------- resident tiles (DMAs staged per phase) -------
            # K-path first so the first matmul isn't stuck behind bulk loads
            coskq_t = res.tile([128, 512], BF16)
            nc.sync.dma_start(out=coskq_t, in_=cosk[:, :])
            sinkq_t = res.tile([128, 512], BF16)
            nc.sync.dma_start(out=sinkq_t, in_=sink[:, :])
            pswap_t = res.tile([128, 128], BF16)
            nc.sync.dma_start(out=pswap_t, in_=pswap[:, :])

            # allocated here (tag order: qts before outu_a), loaded later
            qts = res.tile([128, NIC, RPC], BF16)

            qhs = res.tile([128, H, RPC], BF16)     # rope'd q, [dh, h, rows]
            khs = res.tile([128, HKV, S], BF16)     # rope'd k, [dh, hk, keys]
            vhs = res.tile([128, 16, DKV], BF16)    # v heads, [tok%128, tokc, kv]
            # outu_a shares qts's slot: qts is dead once phase A finishes.
            # split 12/4 so phase D's early matmuls (h<12) don't dep-chain
            # behind the last normalization group (h>=12).
            outu_a = res.tile([128, 12, RPC], BF16, tag="qts")
            outu_b = res.tile([128, 4, RPC], BF16)

            def outu(h):
                return outu_a[:, h, :] if h < 12 else outu_b[:, h - 12, :]
            rec_dram = dramp.tile([16, RPC], F32)
            khs_own = res.tile([128, HKV, 512], BF16)
            vhs_own = res.tile([128, 4, DKV], BF16)
            k_own = dramp.tile([128, HKV, 512], BF16)
            v_own = dramp.tile([128, 4, DKV], BF16)
            k_all = dramp.tile([4, 128, HKV, 512], BF16)
            v_all = dramp.tile([4, 128, 4, DKV], BF16)

            def rope(dst, x_bf, ps_pool, cos_ap, sin_ap, n):
                """dst = x*cos + pairswap(x)*sin  (signs baked into sin)."""
                y_ps = ps_pool.tile([128, 512], F32, tag="mm")
                # moving operand max 1024 bf16 per matmul
                assert n <= 512
                nc.tensor.matmul(y_ps[:, :n], pswap_t, x_bf, start=True,
                                 stop=True)
                t1 = small.tile([128, 512], BF16, tag="t1")
                nc.vector.tensor_mul(t1[:, :n], x_bf, cos_ap)
                t2 = small.tile([128, 512], BF16, tag="t2")
                nc.vector.tensor_mul(t2[:, :n], y_ps[:, :n], sin_ap)
                nc.vector.tensor_add(dst, t1[:, :n], t2[:, :n])

            # ------- Phase B1: K proj for OWN 512-token quarter + RoPE -----
            # (first, so the K all-gather overlaps V proj + Q proj below)
            # wk + kmov loaded in interleaved 0.5MB chunks so the first
            # matmuls start after ~1MB instead of after the full 4MB.
            # Later-phase loads are issued from the Vector queue mid-K-proj
            # so the K path gets the full DMA bandwidth at t=0.
            kmov_a = stream2m.tile([128, NIC // 2, 512], BF16, tag="s2m",
                                   name="kmov_a")
            kmov_b = stream2m.tile([128, NIC // 2, 512], BF16, tag="s2m",
                                   name="kmov_b")

            def kmov(ic):
                return kmov_a[:, ic, :] if ic < 8 else kmov_b[:, ic - 8, :]
            wks = res.tile([128, HKV, NIC * 128], BF16)
            for c in range(4):
                nc.sync.dma_start(out=wks[:, c, :], in_=wk[c])
                half = kmov_a if c < 2 else kmov_b
                nc.sync.dma_start(
                    out=half[:, 4 * (c % 2):4 * (c % 2) + 4, :],
                    in_=kt[:, 4 * c * 512:(4 * c + 4) * 512].rearrange(
                        "p (i m) -> p i m", i=4))
            ehot_t = res.tile([128, 16], BF16)
            nc.sync.dma_start(out=ehot_t, in_=ehot[:, :])
            # touch Exp early so the ~2.7us ACT table load is off the
            # attention critical path
            warm = res.tile([1, 16], BF16)
            nc.scalar.activation(warm, ehot_t[:1, :],
                                 mybir.ActivationFunctionType.Exp)
            wvs = res.tile([128, NIC, DKV], BF16)
            vmovs = []
            for j in range(4):
                vmov = stream05.tile([128, NIC, 128], BF16, tag="s05",
                                     name="vmov")
                vmovs.append(vmov)
            cosq_t = res.tile([128, RPC], BF16)
            sinq_t = res.tile([128, RPC], BF16)
            wq_tiles = []
            for oc in range(1):
                wq_all = stream05.tile([128, NIC, 128], BF16, tag="s05",
                                       name="wq_all")
                wq_tiles.append(wq_all)
            # cos/sin for own k-token quarter live in coskq (host-sliced)
            # rope for hk runs after hk+1's matmuls so the rope matmul
            # (which waits on the ScalarE copy) never blocks the PE FIFO
            pend_k = None
            for hk in range(HKV):
                wk_all = wks[:, hk, :].rearrange("p (i m) -> p i m", i=NIC)
                ps = psmm.tile([128, 512], F32, tag="mm")
                for ic in range(NIC):
                    nc.tensor.matmul(ps, wk_all[:, ic, :],
                                     kmov(ic),
                                     start=(ic == 0), stop=(ic == NIC - 1))
                xk = small.tile([128, 512], BF16, tag="xq")
                nc.scalar.copy(xk, ps)
                if hk == 1:
                    # V-path loads, issued once the K path is streaming
                    nc.scalar.dma_start(out=wvs, in_=wv[:, :].rearrange(
                        "p (i n) -> p i n", i=NIC))
                    for j in range(4):
                        nc.scalar.dma_start(out=vmovs[j], in_=vt[j].rearrange(
                            "p (i m) -> p i m", i=NIC))
                elif hk == 3:
                    # Q-path loads
                    for c in range(4):
                        nc.scalar.dma_start(
                            out=qts[:, 4 * c:4 * c + 4, :],
                            in_=qt[:, 4 * c * 512:(4 * c + 4) * 512]
                            .rearrange("p (i m) -> p i m", i=4))
                    nc.scalar.dma_start(out=cosq_t, in_=cosq[:, :])
                    nc.scalar.dma_start(out=sinq_t, in_=sinq[:, :])
                    for oc in range(1):
                        nc.scalar.dma_start(
                            out=wq_tiles[oc], in_=wq[oc].rearrange(
                                "p (i m) -> p i m", i=NIC))
                if pend_k is not None:
                    rope(khs_own[:, pend_k[0], :], pend_k[1], psmm,
                         coskq_t, sinkq_t, 512)
                pend_k = (hk, xk)
            rope(khs_own[:, pend_k[0], :], pend_k[1], psmm,
                 coskq_t, sinkq_t, 512)

            # ---- all-gather projected K across the 4 cores of the batch --
            nc.sync.dma_start(out=k_own, in_=khs_own)
            nc.gpsimd.collective_compute(
                "AllGather", mybir.AluOpType.bypass,
                replica_groups=[[0, 1, 2, 3], [4, 5, 6, 7]],
                ins=[k_own[:, :, :]], outs=[k_all[:, :, :, :]])
            # land gathered K via the otherwise-idle gpsimd queue: these
            # wait on the collective and must not HOL-block input loads
            for r in range(4):
                nc.gpsimd.dma_start(out=khs[:, :, r * 512:(r + 1) * 512],
                                    in_=k_all[r])

            # ------- Phase B2: V proj for OWN quarter + all-gather ---------
            for j in range(4):            # own 128-token blocks (V stationary)
                vmov = vmovs[j]
                ps = psmm.tile([128, 512], F32, tag="mm")
                for ic in range(NIC):
                    nc.tensor.matmul(ps, vmov[:, ic, :],
                                     wvs[:, ic, :],
                                     start=(ic == 0), stop=(ic == NIC - 1))
                nc.vector.tensor_copy(vhs_own[:, j, :], ps)

            nc.sync.dma_start(out=v_own, in_=vhs_own)
            nc.gpsimd.collective_compute(
                "AllGather", mybir.AluOpType.bypass,
                replica_groups=[[0, 1, 2, 3], [4, 5, 6, 7]],
                ins=[v_own[:, :, :]], outs=[v_all[:, :, :, :]])
            for r in range(4):
                nc.gpsimd.dma_start(out=vhs[:, 4 * r:4 * r + 4, :],
                                    in_=v_all[r])

            # ---------------- Phase A: Q-proj + RoPE ----------------
            pend_q = None
            for oc in range(H):
                if oc < 1:
                    wq_all = wq_tiles[oc]
                else:
                    wq_all = stream05.tile([128, NIC, 128], BF16, tag="s05")
                    nc.sync.dma_start(out=wq_all, in_=wq[oc].rearrange(
                        "p (i m) -> p i m", i=NIC))
                ps = psmm.tile([128, 512], F32, tag="mm")
                for ic in range(NIC):
                    nc.tensor.matmul(ps, wq_all[:, ic, :],
                                     qts[:, ic, :],
                                     start=(ic == 0), stop=(ic == NIC - 1))
                xq = small.tile([128, 512], BF16, tag="xq")
                nc.scalar.copy(xq, ps)
                if pend_q is not None:
                    rope(qhs[:, pend_q[0], :], pend_q[1], psmm,
                         cosq_t, sinq_t, RPC)
                pend_q = (oc, xq)
            rope(qhs[:, pend_q[0], :], pend_q[1], psmm, cosq_t, sinq_t, RPC)

            # ---------------- Phase C: attention, 2 heads at a time -------
            if mask_mul:
                mwid = 128 if mode == "causal" else RPC
                m01s = res.tile([128, NKC, mwid], BF16)
                nc.sync.dma_start(out=m01s, in_=m01[:, :].rearrange(
                    "p (k m) -> p k m", k=NKC))

            def normalize_group(g, sm):
                """reciprocal + broadcast + in-place normalize for the 4
                heads of group g, given their sums in SBUF sm [4, RPC]."""
                rec = small.tile([4, RPC], F32, tag="rec", bufs=2)
                nc.vector.reciprocal_approx_fast(rec, sm)
                nc.sync.dma_start(out=rec_dram[4 * g:4 * g + 4, :], in_=rec)
                for j in range(4):
                    h = 4 * g + j
                    recb = bcastp.tile([128, RPC], F32, tag="bc")
                    nc.sync.dma_start(
                        out=recb,
                        in_=rec_dram[h:h + 1, :].to_broadcast([128, RPC]))
                    nc.vector.tensor_mul(outu(h), outu(h), recb)

            for g in range(HKV):
                hk = g
                ps_s = pssum.tile([4, 512], F32, tag="sum")
                for pair in range(2):
                    h0 = 4 * g + 2 * pair
                    ps_o0 = psacc.tile([128, 512], F32, tag="acc")
                    ps_o1 = psacc.tile([128, 512], F32, tag="acc2")

                    def av(kc, probs):
                        n = n_list[kc]
                        lo = RPC - n
                        for j in range(2):
                            nc.tensor.matmul(
                                (ps_o0 if j == 0 else ps_o1)[:, lo:],
                                vhs[:, kc, hk * 128:(hk + 1) * 128],
                                probs[:, j, :n],
                                start=(kc == 0), stop=(kc == NKC - 1),
                                skip_group_check=True)

                    # AV for key tile kc issues after kc+1's scores so the
                    # PE never waits on exp/mask of the current tile.
                    # Row sums: probs of each 4-tile band (equal widths) are
                    # pre-summed on VectorE; the ones-matmuls then run once
                    # per band at the end of the pair (4x fewer PE columns).
                    pend = None
                    sbands = []
                    bprobs = []
                    for kc in range(NKC):
                        n = n_list[kc]
                        lo = RPC - n          # suffix columns
                        probs = probsp.tile([128, 2, 512], BF16, tag="pr")
                        if mode == "causal" and n <= 256:
                            # both heads' scores packed into one PSUM bank
                            ps_sc = psmm.tile([128, 512], F32, tag="mm")
                            for j in range(2):
                                nc.tensor.matmul(
                                    ps_sc[:, j * n:(j + 1) * n],
                                    khs[:, hk, kc * 128:(kc + 1) * 128],
                                    qhs[:, h0 + j, lo:],
                                    start=(j == 0), stop=(j == 1),
                                    skip_group_check=True)
                            nc.scalar.activation(
                                probs[:, :, :n],
                                ps_sc[:, :2 * n].rearrange(
                                    "p (j n) -> p j n", j=2),
                                mybir.ActivationFunctionType.Exp, scale=SCALE)
                        else:
                            for j in range(2):
                                ps_sc = psmm.tile([128, 512], F32, tag="mm")
                                nc.tensor.matmul(
                                    ps_sc[:, :n],
                                    khs[:, hk, kc * 128:(kc + 1) * 128],
                                    qhs[:, h0 + j, lo:],
                                    start=True, stop=True,
                                    skip_group_check=True)
                                nc.scalar.activation(
                                    probs[:, j, :n], ps_sc[:, :n],
                                    mybir.ActivationFunctionType.Exp,
                                    scale=SCALE)
                        if mask_mul:
                            if mode == "causal":
                                # only the lowest 128-q block of the suffix
                                # can be masked (tril diagonal or all-zero)
                                nc.vector.tensor_mul(
                                    probs[:, :, :128], probs[:, :, :128],
                                    m01s[:, kc:kc + 1, :]
                                    .to_broadcast([128, 2, 128]))
                            else:
                                for j in range(2):
                                    nc.vector.tensor_mul(
                                        probs[:, j, :n], probs[:, j, :n],
                                        m01s[:, kc, lo:])
                        if pend is not None:
                            av(*pend)
                        pend = (kc, probs)
                        bprobs.append(probs)
                        if len(bprobs) == 4:
                            sb = probsp.tile([128, 2, 512], BF16, tag="sb",
                                             bufs=4, name="sb")
                            nc.vector.tensor_add(sb[:, :, :n],
                                                 bprobs[0][:, :, :n],
                                                 bprobs[1][:, :, :n])
                            nc.vector.tensor_add(sb[:, :, :n], sb[:, :, :n],
                                                 bprobs[2][:, :, :n])
                            nc.vector.tensor_add(sb[:, :, :n], sb[:, :, :n],
                                                 bprobs[3][:, :, :n])
                            sbands.append((n, sb))
                            bprobs = []
                    av(*pend)
                    for b, (n, sb) in enumerate(sbands):
                        lo = RPC - n
                        first = b == 0 and pair == 0
                        last = b == len(sbands) - 1 and pair == 1
                        for j in range(2):
                            h = h0 + j
                            nc.tensor.matmul(
                                ps_s[:, lo:], ehot_t[:, 4 * (h - 4 * g):
                                                     4 * (h - 4 * g) + 4],
                                sb[:, j, :n],
                                start=(first and j == 0),
                                stop=(last and j == 1),
                                skip_group_check=True)
                    nc.vector.tensor_copy(outu(h0), ps_o0)
                    nc.vector.tensor_copy(outu(h0 + 1), ps_o1)
                sm = small.tile([4, RPC], F32, tag="sm4", bufs=2)
                nc.vector.tensor_copy(sm, ps_s)
                normalize_group(g, sm)

            # ---------------- Phase D: out-projection ----------------
            for oc in range(4):
                wo_a = stream2m.tile([128, H // 2, 512], BF16, tag="s2m",
                                     name="wo_a")
                nc.sync.dma_start(out=wo_a, in_=wo[oc, :, :H // 2 * 512]
                                  .rearrange("p (h m) -> p h m", h=H // 2))
                wo_b = stream2m.tile([128, H // 2, 512], BF16, tag="s2m",
                                     name="wo_b")
                nc.sync.dma_start(out=wo_b, in_=wo[oc, :, H // 2 * 512:]
                                  .rearrange("p (h m) -> p h m", h=H // 2))

                def wo_all(h):
                    return wo_a[:, h, :] if h < 8 else wo_b[:, h - 8, :]
                for qc in range(NQB):
                    if qc % 2:
                        ps_f = psmm.tile([128, 512], F32, tag="mm",
                                         name="ps_f")
                    else:
                        ps_f = psacc.tile([128, 512], F32, tag="acc",
                                          name="ps_f")
                    for h in range(H):
                        lh = outu_a[:, h, qc * 128:(qc + 1) * 128] if h < 12 \
                            else outu_b[:, h - 12, qc * 128:(qc + 1) * 128]
                        nc.tensor.matmul(
                            ps_f, lh, wo_all(h),
                            start=(h == 0), stop=(h == H - 1))
                    fin = small.tile([128, 512], BF16, tag="fin")
                    nc.vector.tensor_copy(fin, ps_f)
                    nc.sync.dma_start(
                        out=out[qc * 128:(qc + 1) * 128,
                                oc * 512:(oc + 1) * 512],
                        in_=fin)

    nc.compile()
    return nc


def _get_nc(mode: str):
    if mode not in _NC_CACHE:
        _NC_CACHE[mode] = _build(mode)
    return _NC_CACHE[mode]


def _core_rows(mode: str, r: int) -> np.ndarray:
    """Global (within-batch) q-row indices owned by quarter r, ascending."""
    if mode == "causal":
        blocks = sorted([r, 7 - r, 8 + r, 15 - r])
    else:
        blocks = [4 * r, 4 * r + 1, 4 * r + 2, 4 * r + 3]
    return np.concatenate([np.arange(b * 128, (b + 1) * 128) for b in blocks])


def kernel(q, k, v, mask, freqs, W_q, W_k, W_v, W_o):
    q = np.asarray(q, dtype=np.float32)
    k = np.asarray(k, dtype=np.float32)
    v = np.asarray(v, dtype=np.float32)
    mask = np.asarray(mask, dtype=np.float32)
    freqs = np.asarray(freqs, dtype=np.float32)
    W_q = np.asarray(W_q, dtype=np.float32)
    W_k = np.asarray(W_k, dtype=np.float32)
    W_v = np.asarray(W_v, dtype=np.float32)
    W_o = np.asarray(W_o, dtype=np.float32)

    # ---- mask mode detection ----
    nz = mask != 0
    if nz.all():
        mode = "none"
    else:
        tril = np.tril(np.ones((S, S), dtype=bool))
        mode = "causal" if all(np.array_equal(nz[b], tril) for b in range(B)) \
            else "mask"

    # ---- shared host precomputation ----
    c_full = np.cos(freqs)                      # [S, 64]
    s_full = np.sin(freqs)
    sgn = np.tile(np.array([-1.0, 1.0], np.float32), DH // 2)  # [-,+,-,+...]
    cosk_h = np.repeat(c_full, 2, axis=1).T.astype(BF)          # [128, S]
    sink_h = (np.repeat(s_full, 2, axis=1) * sgn).T.astype(BF)

    psw = np.zeros((128, 128), np.float32)
    idx = np.arange(128)
    psw[idx, idx ^ 1] = 1.0
    psw = psw.astype(BF)

    eh = np.zeros((128, 16), np.float32)
    for h in range(4):
        eh[:, 4 * h + h] = 1.0
    eh = eh.astype(BF)

    # weight layouts
    # wq[oc, p, i*128+m] = W_q[oc*128+m, i*128+p]
    wq_h = np.ascontiguousarray(
        W_q.reshape(H, 128, NIC, 128).transpose(0, 3, 2, 1)
        .reshape(H, 128, D)).astype(BF)
    # wk[hk, p, i*128+m] = W_k[hk*128+m, i*128+p]
    wk_h = np.ascontiguousarray(
        W_k.reshape(HKV, 128, NIC, 128).transpose(0, 3, 2, 1)
        .reshape(HKV, 128, D)).astype(BF)
    # wv[p, i*512+n] = W_v[n, i*128+p]
    wv_h = np.ascontiguousarray(
        W_v.reshape(DKV, NIC, 128).transpose(2, 1, 0).reshape(128, NIC * DKV)
    ).astype(BF)
    # wo[oc, p, h*512+m] = W_o[oc*512+m, h*128+p]
    wo_h = np.ascontiguousarray(
        W_o.reshape(4, 512, H, 128).transpose(0, 3, 2, 1).reshape(4, 128, -1)
    ).astype(BF)

    # k/v: each core only gets its own 512-token quarter (gathered on device)
    # kt[p, i*512+t] = k[b, tq*512+t, i*128+p] for quarter tq
    kt_b = []   # [B][4] quarters
    vt_b = []
    for b in range(B):
        kt_b.append([np.ascontiguousarray(
            k[b, tq * 512:(tq + 1) * 512].reshape(512, NIC, 128)
            .transpose(2, 1, 0).reshape(128, NIC * 512)).astype(BF)
            for tq in range(4)])
        # vt[j, p, i*128+t] = v[b, tq*512 + j*128+t, i*128+p]
        vt_b.append([np.ascontiguousarray(
            v[b, tq * 512:(tq + 1) * 512].reshape(4, 128, NIC, 128)
            .transpose(0, 3, 2, 1).reshape(4, 128, NIC * 128)).astype(BF)
            for tq in range(4)])

    in_maps = []
    rows_all = []
    for c in range(NCORES):
        b, r = divmod(c, 4)
        rows = _core_rows(mode, r)
        rows_all.append((b, rows))
        # qt[p, i*512+t] = q[b, rows[t], i*128+p]
        qsl = q[b][rows]                       # [512, D]
        qt_h = np.ascontiguousarray(
            qsl.reshape(RPC, NIC, 128).transpose(2, 1, 0).reshape(128, -1)
        ).astype(BF)
        cq = np.repeat(c_full[rows], 2, axis=1).T.astype(BF)      # [128, 512]
        sq = (np.repeat(s_full[rows], 2, axis=1) * sgn).T.astype(BF)
        im = {
            "wq": wq_h, "qt": qt_h, "kt": kt_b[b][r], "vt": vt_b[b][r],
            "wk": wk_h, "wv": wv_h, "wo": wo_h,
            "cosq": cq, "sinq": sq,
            "cosk": np.ascontiguousarray(cosk_h[:, r * 512:(r + 1) * 512]),
            "sink": np.ascontiguousarray(sink_h[:, r * 512:(r + 1) * 512]),
            "pswap": psw, "ehot": eh,
        }
        if mode == "causal":
            # narrow mask: per key tile kc, the 0/1 mask of the LOWEST
            # 128-q block of this core's kept suffix (tril, zero, or ones)
            n_l = _n_list(mode)
            m01_h = np.empty((NKC, 128, 128), np.float32)
            for kc in range(NKC):
                lo = RPC - n_l[kc]
                qrows = rows[lo:lo + 128]          # global q rows of block
                kcols = np.arange(kc * 128, (kc + 1) * 128)
                m01_h[kc] = (qrows[None, :] >= kcols[:, None])  # [k, q]
            im["m01"] = np.ascontiguousarray(
                m01_h.transpose(1, 0, 2).reshape(128, -1)).astype(BF)
        elif mode == "mask":
            # m01[p, kc*512+m] = (mask[b, rows[m], kc*128+p] != 0)
            msl = nz[b][rows]                  # [512, S] bool
            m01_h = np.ascontiguousarray(
                msl.T.reshape(NKC, 128, RPC).transpose(1, 0, 2)
                .reshape(128, -1)).astype(BF)
            im["m01"] = m01_h
        in_maps.append(im)

    nc = _get_nc(mode)
    kwargs = {}
    if TRACE:
        kwargs["trace"] = True
        if TRACE_CORES:
            kwargs["trace_cores"] = list(TRACE_CORES)
    results = run_bass_kernel_spmd(nc, in_maps, core_ids=list(range(NCORES)),
                                   **kwargs)
    global LAST_RESULTS
    LAST_RESULTS = results

    full = np.empty((B, S, D), np.float32)
    for c in range(NCORES):
        b, rows = rows_all[c]
        full[b, rows] = results.results[c]["out"].astype(np.float32)
    return full
